# revision 27
# baseline (speedup 1.0000x reference)
"""Multi-head causal attention on 8 Trainium2 NeuronCores.

Problem: x [2, 2048, 1024] f32; Wq/Wk/Wv [1024, 1024]; Wo [1024, 1024]; bo [1024].
  q/k/v = split_heads(x @ W*)  (16 heads, head_dim 64)
  scores = q k^T, causal mask, / sqrt(1024), softmax, out = (w v) @ Wo + bo

Sharding: tensor-parallel over heads, 8-way (Megatron-style): core c computes
heads {2c, 2c+1} for both batches, exchanges attention outputs with two 8-rank
AllToAlls (one per local head), applies the full out_proj on its gathered
[1024, 512] attn^T slice, and returns rows (batch c//4, 512*(c%4):+512).

Host-side prep: x is transposed and cast to bf16 on the host (layout
[B, KT, 128, S]), weights are pre-tiled bf16 — the device kernel does zero
transposes and zero dtype casts of inputs.

On-chip: attention is computed fully transposed (scores^T = K Q^T in [k, q]
layout). The two BATCHES are processed as concurrent PE row-tiles
(tile_position (0,0) / (64,0)) since head_dim=64 is half the array: batch-0
q/k live in partitions 0:64, batch-1 in 64:128, so each matmul pair runs at
full PE rate. Heads run sequentially so head 0's AllToAll overlaps head 1's
attention. Causal diagonal blocks are column-sliced (no wasted exp/matmul on
fully-masked columns). The softmax denominator rides as a 65th column of V
(ones-column trick); normalization uses a gpsimd partition-broadcast.
"""

from contextlib import ExitStack

import numpy as np
import ml_dtypes

import concourse.bass as bass
import concourse.tile as tile
from concourse import bacc, mybir
from concourse.bass_utils import run_bass_kernel_spmd
from concourse.masks import make_identity

F32 = mybir.dt.float32
BF16 = mybir.dt.bfloat16

N_CORES = 8
B = 2
S = 2048
D = 1024
H = 16
DH = 64
H_PER = 2              # heads per core
DCOL = H_PER * DH      # 128: projection output cols per core
KT = D // 128          # 8 contraction tiles
NSP = S // 512         # 4 q-spans per batch
S_SLICE = S // 4       # 512 output rows per core
SCALE = 1.0 / np.sqrt(np.float32(D))
DEPTH = 4              # attention software-pipeline depth

_CACHE = {}
DEBUG = False


def build():
    """Build the SPMD program (identical on all 8 cores)."""
    nc = bacc.Bacc("TRN2", target_bir_lowering=False, debug=False)

    xt_t = nc.dram_tensor("xt", [B, KT, 128, S], BF16, kind="ExternalInput")
    wq_t = nc.dram_tensor("wq", [128, KT, DCOL], BF16, kind="ExternalInput")
    wk_t = nc.dram_tensor("wk", [128, KT, DCOL], BF16, kind="ExternalInput")
    wv_t = nc.dram_tensor("wv", [128, KT, DCOL], BF16, kind="ExternalInput")
    wo_t = nc.dram_tensor("wo", [128, KT, D], BF16, kind="ExternalInput")
    bo_t = nc.dram_tensor("bo", [1, D], F32, kind="ExternalInput")
    out_t = nc.dram_tensor("out", [S_SLICE, D], F32, kind="ExternalOutput")

    if DEBUG:
        dbg = {
            "dbg_qt": nc.dram_tensor("dbg_qt", [128, S], BF16,
                                     kind="ExternalOutput"),
            "dbg_kt": nc.dram_tensor("dbg_kt", [128, S], BF16,
                                     kind="ExternalOutput"),
            "dbg_vp": nc.dram_tensor("dbg_vp", [128, S // 128 * H_PER * (DH + 1)],
                                     BF16, kind="ExternalOutput"),
            "dbg_at": nc.dram_tensor("dbg_at", [128, S], BF16,
                                     kind="ExternalOutput"),
            "dbg_w": nc.dram_tensor("dbg_w", [128, 2 * 512], BF16,
                                    kind="ExternalOutput"),
            "dbg_gsb": nc.dram_tensor("dbg_gsb", [128, KT * 512], BF16,
                                      kind="ExternalOutput"),
            "dbg_o": nc.dram_tensor("dbg_o", [DH + 1, 512], F32,
                                    kind="ExternalOutput"),
            "dbg_rb": nc.dram_tensor("dbg_rb", [DH, 512], F32,
                                     kind="ExternalOutput"),
        }

    warm_in = nc.dram_tensor("warm_in", [8, 16], F32)
    warm_out = nc.dram_tensor("warm_out", [8, 16], F32)
    a2a_in = [
        nc.dram_tensor(f"a2a_in{h}", [8, DH, 512], BF16) for h in range(H_PER)
    ]
    a2a_out = [
        nc.dram_tensor(f"a2a_out{h}", [8, DH, 512], BF16) for h in range(H_PER)
    ]

    with tile.TileContext(nc) as tc, ExitStack() as ctx:
        const = ctx.enter_context(tc.tile_pool(name="const", bufs=1))
        persist = ctx.enter_context(tc.tile_pool(name="persist", bufs=1))
        vstage = ctx.enter_context(tc.tile_pool(name="vstage", bufs=2))
        wpool = ctx.enter_context(tc.tile_pool(name="wpool", bufs=6))
        npool = ctx.enter_context(tc.tile_pool(name="npool", bufs=2))
        opool = ctx.enter_context(tc.tile_pool(name="opool", bufs=3))
        ps_s = ctx.enter_context(tc.tile_pool(name="ps_s", bufs=2, space="PSUM"))
        ps_o = ctx.enter_context(tc.tile_pool(name="ps_o", bufs=2, space="PSUM"))
        ps_v = ctx.enter_context(tc.tile_pool(name="ps_v", bufs=1, space="PSUM"))

        # warmup collective: absorbs the per-execution ncfw entry cost
        # concurrently with the load/projection phase.
        nc.gpsimd.collective_compute(
            "AllToAll", mybir.AluOpType.bypass,
            replica_groups=[list(range(8))],
            ins=[warm_in.ap().opt()], outs=[warm_out.ap().opt()],
        )

        identity = const.tile([128, 128], BF16)
        make_identity(nc, identity)

        # persistent tiles -------------------------------------------------
        xts = [persist.tile([128, KT, S], BF16, tag=f"xts{b}", name=f"xts{b}")
               for b in range(B)]
        # q^T/k^T per head: rows 0:64 = batch 0, rows 64:128 = batch 1
        qTh = [persist.tile([128, S], BF16, tag=f"qT{h}", name=f"qT{h}")
               for h in range(H_PER)]
        kTh = [persist.tile([128, S], BF16, tag=f"kT{h}", name=f"kT{h}")
               for h in range(H_PER)]
        # v per batch: [kpos-part, sb, head, 65] (col 64 = ones -> denom)
        vp = [persist.tile([128, S // 128, H_PER, DH + 1], BF16,
                           tag=f"vp{b}", name=f"vp{b}") for b in range(B)]
        # attn^T per head: rows 0:64 = batch 0, 64:128 = batch 1
        attnT = [persist.tile([128, S], BF16, tag=f"attnT{h}",
                              name=f"attnT{h}") for h in range(H_PER)]
        wq_bf = persist.tile([128, KT, DCOL], BF16, tag="wq", name="wq")
        wk_bf = persist.tile([128, KT, DCOL], BF16, tag="wk", name="wk")
        wv_bf = persist.tile([128, KT, DCOL], BF16, tag="wv", name="wv")
        wo_bf = persist.tile([128, KT, D], BF16, tag="wo", name="wo")
        bias_b = persist.tile([128, D], F32, tag="bias", name="bias")
        g_sb = persist.tile([128, KT, 512], BF16, tag="g", name="g_sb")

        for b in range(B):
            nc.gpsimd.memset(vp[b][:, :, :, DH:], 1.0)

        # input DMAs -------------------------------------------------------
        nc.sync.dma_start(out=wq_bf, in_=wq_t.ap())
        nc.sync.dma_start(out=wk_bf, in_=wk_t.ap())
        nc.sync.dma_start(out=wv_bf, in_=wv_t.ap())
        # x^T chunks: 2KB per-partition lines (1024 tokens) for full DMA
        # bandwidth, span-pair-major so early spans land first
        for sp in range(2):
            for b in range(B):
                for kt in range(KT):
                    nc.sync.dma_start(
                        out=xts[b][:, kt, sp * 1024:(sp + 1) * 1024],
                        in_=xt_t[b, kt, :, sp * 1024:(sp + 1) * 1024])
        nc.sync.dma_start(out=wo_bf, in_=wo_t.ap())
        nc.sync.dma_start(out=bias_b, in_=bo_t[0:1, :].to_broadcast([128, D]))

        # projections ------------------------------------------------------
        def proj_qk(s, b, w_bf, dest):
            """One Q-or-K projection for (span s, batch b). psum rows
            0:64 = head 0, 64:128 = head 1; evicted into the per-head
            tiles at the batch's partition half."""
            cols = slice(s * 512, (s + 1) * 512)
            ps = ps_v.tile([128, 512], F32, tag="qk", name="qk_ps")
            for kt in range(KT):
                nc.tensor.matmul(
                    ps, lhsT=w_bf[:, kt, :], rhs=xts[b][:, kt, cols],
                    start=(kt == 0), stop=(kt == KT - 1))
            for h in range(H_PER):
                nc.vector.tensor_copy(
                    out=dest[h][64 * b:64 * b + 64, cols],
                    in_=ps[64 * h:64 * h + 64, :])

        def proj_v(s, b):
            """V^T projection + PE-transpose into AV layout."""
            cols = slice(s * 512, (s + 1) * 512)
            ps = ps_v.tile([128, 512], F32, tag="qk", name="vt_ps")
            for kt in range(KT):
                nc.tensor.matmul(
                    ps, lhsT=wv_bf[:, kt, :], rhs=xts[b][:, kt, cols],
                    start=(kt == 0), stop=(kt == KT - 1))
            vT = vstage.tile([128, 512], BF16, tag="vT", name="vT")
            nc.vector.tensor_copy(out=vT, in_=ps)
            pt = ps_v.tile([128, 4, 128], BF16, tag="pt", name="pt")
            for c in range(4):
                nc.tensor.transpose(
                    pt[:, c, :], vT[:, c * 128:(c + 1) * 128], identity)
            nc.vector.tensor_copy(
                out=vp[b][:, s * 4:s * 4 + 4, :, 0:DH],
                in_=pt.rearrange("p c (h d) -> p c h d", d=DH))

        def proj_units(s):
            return [
                lambda b=b: proj_qk(s, b, wq_bf, qTh) for b in range(B)
            ] + [
                lambda b=b: proj_qk(s, b, wk_bf, kTh) for b in range(B)
            ] + [
                lambda b=b: proj_v(s, b) for b in range(B)
            ]

        # attention --------------------------------------------------------
        def attention_span(h, s, post_done):
            """(emit_scores, emit_av) closure pairs for (head h, span s),
            batches 0/1 as concurrent PE row-tiles."""
            if True:
                last = 4 * s + 3
                o_ps = [ps_o.tile([DH + 1, 512], F32, tag="o", name=f"o{b}")
                        for b in range(B)]

                def mk_scores(s=s, kb=None):
                    def emit(kb=kb):
                        off = max(0, 128 * (kb - 4 * s))
                        sp = ps_s.tile([128, 2, 512], F32, tag="mm",
                                       name="s_ps")
                        for b in range(B):
                            nc.tensor.matmul(
                                sp[:, b, off:512],
                                lhsT=kTh[h][64 * b:64 * b + 64,
                                            kb * 128:(kb + 1) * 128],
                                rhs=qTh[h][64 * b:64 * b + 64,
                                           s * 512 + off:(s + 1) * 512],
                                start=True, stop=True)
                        w_bf = wpool.tile([128, 2, 512], BF16, tag="w",
                                          name="w_bf")
                        nc.scalar.activation(
                            w_bf[:, :, off:512], sp[:, :, off:512],
                            mybir.ActivationFunctionType.Exp,
                            scale=float(SCALE))
                        if DEBUG and h == 0 and s == 1 and kb == 0:
                            nc.sync.dma_start(
                                out=dbg["dbg_w"].ap(),
                                in_=w_bf.rearrange("p a c -> p (a c)"))
                        if kb >= 4 * s:
                            # causal: inside the diagonal 128-col block,
                            # keep iff col - kpos >= 0 (both batches at once)
                            nc.gpsimd.affine_select(
                                out=w_bf[:, :, off:off + 128],
                                in_=w_bf[:, :, off:off + 128],
                                pattern=[[0, 2], [1, 128]],
                                compare_op=mybir.AluOpType.is_ge,
                                fill=0.0, base=0, channel_multiplier=-1)
                        return w_bf
                    return emit

                def mk_av(s=s, kb=None, last=last, o_ps=o_ps):
                    def emit(w_bf, kb=kb):
                        off = max(0, 128 * (kb - 4 * s))
                        for b in range(B):
                            nc.tensor.matmul(
                                o_ps[b][:, off:512],
                                lhsT=vp[b][:, kb, h, :],
                                rhs=w_bf[:, b, off:512],
                                start=(kb == 0), stop=(kb == last),
                                skip_group_check=True)
                        if kb == last:
                            normalize(h, s, o_ps)
                            if post_done is not None:
                                post_done()
                    return emit

                for kb in range(last + 1):
                    yield mk_scores(kb=kb), mk_av(kb=kb)

        def normalize(h, s, o_ps):
            """Both batches at once: one reciprocal + one broadcast."""
            den = npool.tile([1, 2, 512], F32, tag="dn", name="den")
            for b in range(B):
                nc.vector.tensor_copy(out=den[:, b, :],
                                      in_=o_ps[b][DH:DH + 1, :])
            recip = npool.tile([1, 2, 512], F32, tag="rc", name="recip")
            nc.vector.reciprocal_approx_fast(out=recip, in_=den)
            rb = npool.tile([DH, 2, 512], F32, tag="rb", name="rb")
            nc.gpsimd.partition_broadcast(
                rb.rearrange("p a c -> p (a c)"),
                recip.rearrange("p a c -> p (a c)"), channels=DH)
            for b in range(B):
                nc.vector.tensor_mul(
                    attnT[h][64 * b:64 * b + 64, s * 512:(s + 1) * 512],
                    o_ps[b][0:DH, :], rb[:, b, :])
                nc.sync.dma_start(
                    out=a2a_in[h][4 * b + s],
                    in_=attnT[h][64 * b:64 * b + 64, s * 512:(s + 1) * 512])

        def exchange(h):
            nc.gpsimd.collective_compute(
                "AllToAll", mybir.AluOpType.bypass,
                replica_groups=[list(range(8))],
                ins=[a2a_in[h].ap().opt()], outs=[a2a_out[h].ap().opt()],
            )

        def gather(h):
            nc.sync.dma_start(
                out=g_sb[h * DH:(h + 1) * DH, :, :],
                in_=a2a_out[h].ap().rearrange("kt p c -> p kt c"))

        # emission ---------------------------------------------------------
        # Span-major, head 0 first within each span: h0 finishes its last
        # span one full h1-span before the end, so AllToAll(0) overlaps
        # h1's final span. proj(s+1) units are spread through span-s steps.
        def post_h0():
            exchange(0)

        def post_h1():
            exchange(1)

        for b in range(B):
            proj_qk(0, b, wq_bf, qTh)
            proj_qk(0, b, wk_bf, kTh)
            proj_v(0, b)

        # Block order: h0's spans front-loaded so AllToAll(0) completes
        # before attention ends; h1's late spans provide the cover.
        blocks = [(0, 0), (1, 0), (0, 1), (1, 1), (0, 2), (0, 3), (1, 2),
                  (1, 3)]
        # proj(s+1) filler units go into the blocks processed before the
        # first block that needs span s+1.
        fill_plan = {1: (0, 2), 2: (2, 4), 3: (4, 5)}  # span -> block range

        steps = []
        fill_at = {}
        bounds = []
        for (h, s) in blocks:
            base = len(steps)
            post = post_h0 if (h, s) == (0, 3) else (
                post_h1 if (h, s) == (1, 3) else None)
            steps += list(attention_span(h, s, post))
            bounds.append((base, len(steps)))
        for sp, (b0_, b1_) in fill_plan.items():
            lo = bounds[b0_][0]
            hi = bounds[b1_ - 1][1]
            units = proj_units(sp)
            stride = max(1, (hi - lo) // len(units))
            for u, unit in enumerate(units):
                fill_at.setdefault(lo + min(u * stride, hi - lo - 1),
                                   []).append(unit)

        n = len(steps)
        w_box = {}
        last_w = None
        for i in range(n + DEPTH):
            if i < n:
                w_box[i] = steps[i][0]()
                if i == n - 1:
                    last_w = w_box[i]
            for unit in fill_at.get(i, ()):
                unit()
            if i >= DEPTH:
                steps[i - DEPTH][1](w_box.pop(i - DEPTH))

        if DEBUG:
            nc.sync.dma_start(out=dbg["dbg_qt"].ap(), in_=qTh[0])
            nc.sync.dma_start(out=dbg["dbg_kt"].ap(), in_=kTh[0])
            nc.sync.dma_start(
                out=dbg["dbg_vp"].ap(),
                in_=vp[0].rearrange("p a b c -> p (a b c)"))
            nc.sync.dma_start(out=dbg["dbg_at"].ap(), in_=attnT[0])
            nc.sync.dma_start(
                out=dbg["dbg_gsb"].ap(), in_=g_sb.rearrange("p a c -> p (a c)"))

        # ---- tail ---------------------------------------------------------
        # gathers are emitted HERE (not at the exchange) so the out_proj
        # inputs only become ready after attention is fully drained — the
        # priority-heap scheduler would otherwise backfill out_proj matmuls
        # into attention-pipeline bubbles and delay the critical chain.
        gather(0)
        gather(1)

        # warm-A: anchored on the last exp output (ready just before the
        # final AV), bridges the last-span normalize window.
        warm_ps = ps_v.tile([128, 512], F32, tag="qk", name="warm_ps")
        for i in range(24):
            nc.tensor.matmul(
                warm_ps, lhsT=wo_bf[:, i % KT, 0:128],
                rhs=last_w[:, 0, :], start=True, stop=True)

        # out_proj split by head-half (row-tiled K=64): h0 halves need only
        # AllToAll(0) — which completed during h1's last spans — so they run
        # immediately after attention; h1 halves complete each psum once
        # AllToAll(1) lands. warm-B bridges the gap in between.
        op_ps = {}

        def op_half(sb, hh):
            if hh == 0:
                op_ps[sb] = ps_s.tile([128, 2, 512], F32, tag="mm",
                                      name="op_ps")
            ps = op_ps[sb]
            rows = slice(64 * hh, 64 * hh + 64)
            for nt in range(2):
                for kt in range(KT):
                    nc.tensor.matmul(
                        ps[:, nt, :],
                        lhsT=g_sb[rows, kt, sb * 128:(sb + 1) * 128],
                        rhs=wo_bf[rows, kt, nt * 512:(nt + 1) * 512],
                        start=(hh == 0 and kt == 0),
                        stop=(hh == 1 and kt == KT - 1),
                        skip_group_check=True)
            if hh == 1:
                ot = opool.tile([128, D], F32, tag="ot")
                nc.vector.tensor_add(
                    ot, ps.rearrange("p a c -> p (a c)"), bias_b)
                nc.sync.dma_start(
                    out=out_t[sb * 128:(sb + 1) * 128, :], in_=ot)

        op_half(0, 0)
        op_half(1, 0)
        # warm-B: anchored on the final attnT write (ready ~5us after the
        # last AV) so it fills the residual AllToAll(1) wait.
        for i in range(24):
            nc.tensor.matmul(
                warm_ps, lhsT=wo_bf[:, i % KT, 0:128],
                rhs=attnT[1][:, 3 * 512:4 * 512], start=True, stop=True)
        op_half(0, 1)
        op_half(2, 0)
        op_half(1, 1)
        op_half(3, 0)
        op_half(2, 1)
        op_half(3, 1)

    nc.compile()
    return nc


def shard_inputs(x, Wq, Wk, Wv, Wo, bo):
    """Full f32 inputs -> per-core in_maps (host-side bf16 + layout prep)."""
    x = np.asarray(x, dtype=np.float32)
    # x^T per batch, tiled: [B, KT, 128, S]
    xt = np.ascontiguousarray(
        x.transpose(0, 2, 1).reshape(B, KT, 128, S)).astype(ml_dtypes.bfloat16)

    def tile_w(w, cols):
        w = np.asarray(w, dtype=np.float32)[:, cols]
        return np.ascontiguousarray(
            w.reshape(KT, 128, w.shape[1]).transpose(1, 0, 2)
        ).astype(ml_dtypes.bfloat16)

    wo_full = tile_w(Wo, slice(None))
    bo = np.asarray(bo, dtype=np.float32).reshape(1, D)
    in_maps = []
    for c in range(N_CORES):
        cols = slice(c * DCOL, (c + 1) * DCOL)
        in_maps.append({
            "xt": xt,
            "wq": tile_w(Wq, cols),
            "wk": tile_w(Wk, cols),
            "wv": tile_w(Wv, cols),
            "wo": wo_full,
            "bo": bo,
        })
    return in_maps


def assemble_output(results):
    """Per-core out slices -> full [B, S, D]."""
    out = np.empty((B, S, D), dtype=np.float32)
    for c in range(N_CORES):
        b, sl = c // 4, c % 4
        out[b, sl * S_SLICE:(sl + 1) * S_SLICE, :] = results[c]["out"]
    return out


def kernel(x, Wq, Wk, Wv, Wo, bo):
    if "nc" not in _CACHE:
        _CACHE["nc"] = build()
    nc = _CACHE["nc"]
    in_maps = shard_inputs(x, Wq, Wk, Wv, Wo, bo)
    res = run_bass_kernel_spmd(nc, in_maps, core_ids=list(range(N_CORES)))
    return assemble_output(res.results)


# revision 31
# speedup vs baseline: 1.1031x; 1.1031x over previous
"""Multi-head causal attention on 8 Trainium2 NeuronCores.

Problem: x [2, 2048, 1024] f32; Wq/Wk/Wv [1024, 1024]; Wo [1024, 1024]; bo [1024].
  q/k/v = split_heads(x @ W*)  (16 heads, head_dim 64)
  scores = q k^T, causal mask, / sqrt(1024), softmax, out = (w v) @ Wo + bo

Sharding: tensor-parallel over heads, 8-way (Megatron-style): core c computes
heads {2c, 2c+1} for both batches, exchanges attention outputs with two 8-rank
AllToAlls (one per local head), applies the full out_proj on its gathered
[1024, 512] attn^T slice, and returns rows (batch c//4, 512*(c%4):+512).

Host-side prep: x is transposed and cast to bf16 on the host (layout
[B, KT, 128, S]), weights are pre-tiled bf16 — the device kernel does zero
transposes and zero dtype casts of inputs.

On-chip: attention is computed fully transposed (scores^T = K Q^T in [k, q]
layout). The two BATCHES are processed as concurrent PE row-tiles
(tile_position (0,0) / (64,0)) since head_dim=64 is half the array: batch-0
q/k live in partitions 0:64, batch-1 in 64:128, so each matmul pair runs at
full PE rate. Heads run sequentially so head 0's AllToAll overlaps head 1's
attention. Causal diagonal blocks are column-sliced (no wasted exp/matmul on
fully-masked columns). The softmax denominator rides as a 65th column of V
(ones-column trick); normalization uses a gpsimd partition-broadcast.
"""

from contextlib import ExitStack

import numpy as np
import ml_dtypes

import concourse.bass as bass
import concourse.tile as tile
from concourse import bacc, mybir
from concourse.bass_utils import run_bass_kernel_spmd
from concourse.masks import make_identity

F32 = mybir.dt.float32
BF16 = mybir.dt.bfloat16

N_CORES = 8
B = 2
S = 2048
D = 1024
H = 16
DH = 64
H_PER = 2              # heads per core
DCOL = H_PER * DH      # 128: projection output cols per core
KT = D // 128          # 8 contraction tiles
NSP = S // 512         # 4 q-spans per batch
S_SLICE = S // 4       # 512 output rows per core
SCALE = 1.0 / np.sqrt(np.float32(D))
DEPTH = 4              # attention software-pipeline depth

_CACHE = {}
DEBUG = False


def build():
    """Build the SPMD program (identical on all 8 cores)."""
    nc = bacc.Bacc("TRN2", target_bir_lowering=False, debug=False)

    xt_t = nc.dram_tensor("xt", [B, KT, 128, S], BF16, kind="ExternalInput")
    wq_t = nc.dram_tensor("wq", [128, KT, DCOL], BF16, kind="ExternalInput")
    wk_t = nc.dram_tensor("wk", [128, KT, DCOL], BF16, kind="ExternalInput")
    wv_t = nc.dram_tensor("wv", [128, KT, DCOL], BF16, kind="ExternalInput")
    wo_t = nc.dram_tensor("wo", [128, KT, D], BF16, kind="ExternalInput")
    bo_t = nc.dram_tensor("bo", [1, D], F32, kind="ExternalInput")
    out_t = nc.dram_tensor("out", [S_SLICE, D], F32, kind="ExternalOutput")

    if DEBUG:
        dbg = {
            "dbg_qt": nc.dram_tensor("dbg_qt", [128, S], BF16,
                                     kind="ExternalOutput"),
            "dbg_kt": nc.dram_tensor("dbg_kt", [128, S], BF16,
                                     kind="ExternalOutput"),
            "dbg_vp": nc.dram_tensor("dbg_vp", [128, S // 128 * H_PER * (DH + 1)],
                                     BF16, kind="ExternalOutput"),
            "dbg_at": nc.dram_tensor("dbg_at", [128, S], BF16,
                                     kind="ExternalOutput"),
            "dbg_w": nc.dram_tensor("dbg_w", [128, 2 * 512], BF16,
                                    kind="ExternalOutput"),
            "dbg_gsb": nc.dram_tensor("dbg_gsb", [128, KT * 512], BF16,
                                      kind="ExternalOutput"),
            "dbg_o": nc.dram_tensor("dbg_o", [DH + 1, 512], F32,
                                    kind="ExternalOutput"),
            "dbg_rb": nc.dram_tensor("dbg_rb", [DH, 512], F32,
                                     kind="ExternalOutput"),
        }

    warm_in = nc.dram_tensor("warm_in", [8, 16], F32)
    warm_out = nc.dram_tensor("warm_out", [8, 16], F32)
    a2a_in = [
        nc.dram_tensor(f"a2a_in{h}", [8, DH, 512], BF16) for h in range(H_PER)
    ]
    a2a_out = [
        nc.dram_tensor(f"a2a_out{h}", [8, DH, 512], BF16) for h in range(H_PER)
    ]

    with tile.TileContext(nc) as tc, ExitStack() as ctx:
        const = ctx.enter_context(tc.tile_pool(name="const", bufs=1))
        persist = ctx.enter_context(tc.tile_pool(name="persist", bufs=1))
        vstage = ctx.enter_context(tc.tile_pool(name="vstage", bufs=2))
        wpool = ctx.enter_context(tc.tile_pool(name="wpool", bufs=6))
        npool = ctx.enter_context(tc.tile_pool(name="npool", bufs=2))
        opool = ctx.enter_context(tc.tile_pool(name="opool", bufs=3))
        ps_s = ctx.enter_context(tc.tile_pool(name="ps_s", bufs=2, space="PSUM"))
        ps_o = ctx.enter_context(tc.tile_pool(name="ps_o", bufs=2, space="PSUM"))
        ps_v = ctx.enter_context(tc.tile_pool(name="ps_v", bufs=1, space="PSUM"))

        # warmup collective: absorbs the per-execution ncfw entry cost
        # concurrently with the load/projection phase.
        nc.gpsimd.collective_compute(
            "AllToAll", mybir.AluOpType.bypass,
            replica_groups=[list(range(8))],
            ins=[warm_in.ap().opt()], outs=[warm_out.ap().opt()],
        )

        identity = const.tile([128, 128], BF16)
        make_identity(nc, identity)

        # persistent tiles -------------------------------------------------
        xts = [persist.tile([128, KT, S], BF16, tag=f"xts{b}", name=f"xts{b}")
               for b in range(B)]
        # q^T/k^T per head: rows 0:64 = batch 0, rows 64:128 = batch 1
        qTh = [persist.tile([128, S], BF16, tag=f"qT{h}", name=f"qT{h}")
               for h in range(H_PER)]
        kTh = [persist.tile([128, S], BF16, tag=f"kT{h}", name=f"kT{h}")
               for h in range(H_PER)]
        # v per batch: [kpos-part, sb, head, 65] (col 64 = ones -> denom)
        vp = [persist.tile([128, S // 128, H_PER, DH + 1], BF16,
                           tag=f"vp{b}", name=f"vp{b}") for b in range(B)]
        # attn^T per head: rows 0:64 = batch 0, 64:128 = batch 1
        attnT = [persist.tile([128, S], BF16, tag=f"attnT{h}",
                              name=f"attnT{h}") for h in range(H_PER)]
        wq_bf = persist.tile([128, KT, DCOL], BF16, tag="wq", name="wq")
        wk_bf = persist.tile([128, KT, DCOL], BF16, tag="wk", name="wk")
        wv_bf = persist.tile([128, KT, DCOL], BF16, tag="wv", name="wv")
        wo_bf = persist.tile([128, KT, D], BF16, tag="wo", name="wo")
        bias_b = persist.tile([128, D], F32, tag="bias", name="bias")
        g_sb = persist.tile([128, KT, 512], BF16, tag="g", name="g_sb")

        for b in range(B):
            nc.gpsimd.memset(vp[b][:, :, :, DH:], 1.0)

        # input DMAs -------------------------------------------------------
        nc.sync.dma_start(out=wq_bf, in_=wq_t.ap())
        nc.sync.dma_start(out=wk_bf, in_=wk_t.ap())
        nc.sync.dma_start(out=wv_bf, in_=wv_t.ap())
        # x^T chunks: 2KB per-partition lines (1024 tokens) for full DMA
        # bandwidth, span-pair-major so early spans land first
        for sp in range(2):
            for b in range(B):
                for kt in range(KT):
                    nc.sync.dma_start(
                        out=xts[b][:, kt, sp * 1024:(sp + 1) * 1024],
                        in_=xt_t[b, kt, :, sp * 1024:(sp + 1) * 1024])
        nc.sync.dma_start(out=wo_bf, in_=wo_t.ap())
        nc.sync.dma_start(out=bias_b, in_=bo_t[0:1, :].to_broadcast([128, D]))

        # projections ------------------------------------------------------
        def proj_qk(s, b, w_bf, dest):
            """One Q-or-K projection for (span s, batch b). psum rows
            0:64 = head 0, 64:128 = head 1; evicted into the per-head
            tiles at the batch's partition half."""
            cols = slice(s * 512, (s + 1) * 512)
            ps = ps_v.tile([128, 512], F32, tag="qk", name="qk_ps")
            for kt in range(KT):
                nc.tensor.matmul(
                    ps, lhsT=w_bf[:, kt, :], rhs=xts[b][:, kt, cols],
                    start=(kt == 0), stop=(kt == KT - 1))
            for h in range(H_PER):
                nc.vector.tensor_copy(
                    out=dest[h][64 * b:64 * b + 64, cols],
                    in_=ps[64 * h:64 * h + 64, :])

        def proj_v(s, b):
            """V^T projection + PE-transpose into AV layout."""
            cols = slice(s * 512, (s + 1) * 512)
            ps = ps_v.tile([128, 512], F32, tag="qk", name="vt_ps")
            for kt in range(KT):
                nc.tensor.matmul(
                    ps, lhsT=wv_bf[:, kt, :], rhs=xts[b][:, kt, cols],
                    start=(kt == 0), stop=(kt == KT - 1))
            vT = vstage.tile([128, 512], BF16, tag="vT", name="vT")
            nc.vector.tensor_copy(out=vT, in_=ps)
            pt = ps_v.tile([128, 4, 128], BF16, tag="pt", name="pt")
            for c in range(4):
                nc.tensor.transpose(
                    pt[:, c, :], vT[:, c * 128:(c + 1) * 128], identity)
            nc.vector.tensor_copy(
                out=vp[b][:, s * 4:s * 4 + 4, :, 0:DH],
                in_=pt.rearrange("p c (h d) -> p c h d", d=DH))

        def proj_units(s):
            return [
                lambda b=b: proj_qk(s, b, wq_bf, qTh) for b in range(B)
            ] + [
                lambda b=b: proj_qk(s, b, wk_bf, kTh) for b in range(B)
            ] + [
                lambda b=b: proj_v(s, b) for b in range(B)
            ]

        # attention --------------------------------------------------------
        def attention_span(h, s, post_done):
            """(emit_scores, emit_av) closure pairs for (head h, span s),
            batches 0/1 as concurrent PE row-tiles."""
            if True:
                last = 4 * s + 3
                o_ps = [ps_o.tile([DH + 1, 512], F32, tag="o", name=f"o{b}")
                        for b in range(B)]

                def mk_scores(s=s, kb=None):
                    def emit(kb=kb):
                        off = max(0, 128 * (kb - 4 * s))
                        sp = ps_s.tile([128, 2, 512], F32, tag="mm",
                                       name="s_ps")
                        for b in range(B):
                            nc.tensor.matmul(
                                sp[:, b, off:512],
                                lhsT=kTh[h][64 * b:64 * b + 64,
                                            kb * 128:(kb + 1) * 128],
                                rhs=qTh[h][64 * b:64 * b + 64,
                                           s * 512 + off:(s + 1) * 512],
                                start=True, stop=True)
                        w_bf = wpool.tile([128, 2, 512], BF16, tag="w",
                                          name="w_bf")
                        nc.scalar.activation(
                            w_bf[:, :, off:512], sp[:, :, off:512],
                            mybir.ActivationFunctionType.Exp,
                            scale=float(SCALE))
                        if DEBUG and h == 0 and s == 1 and kb == 0:
                            nc.sync.dma_start(
                                out=dbg["dbg_w"].ap(),
                                in_=w_bf.rearrange("p a c -> p (a c)"))
                        if kb >= 4 * s:
                            # causal: inside the diagonal 128-col block,
                            # keep iff col - kpos >= 0 (both batches at once)
                            nc.gpsimd.affine_select(
                                out=w_bf[:, :, off:off + 128],
                                in_=w_bf[:, :, off:off + 128],
                                pattern=[[0, 2], [1, 128]],
                                compare_op=mybir.AluOpType.is_ge,
                                fill=0.0, base=0, channel_multiplier=-1)
                        return w_bf
                    return emit

                def mk_av(s=s, kb=None, last=last, o_ps=o_ps):
                    def emit(w_bf, kb=kb):
                        off = max(0, 128 * (kb - 4 * s))
                        for b in range(B):
                            nc.tensor.matmul(
                                o_ps[b][:, off:512],
                                lhsT=vp[b][:, kb, h, :],
                                rhs=w_bf[:, b, off:512],
                                start=(kb == 0), stop=(kb == last),
                                skip_group_check=True)
                        if kb == last:
                            normalize(h, s, o_ps)
                            if post_done is not None:
                                post_done()
                    return emit

                for kb in range(last + 1):
                    yield mk_scores(kb=kb), mk_av(kb=kb)

        def normalize(h, s, o_ps):
            """Both batches at once: one reciprocal + one broadcast."""
            den = npool.tile([1, 2, 512], F32, tag="dn", name="den")
            for b in range(B):
                nc.vector.tensor_copy(out=den[:, b, :],
                                      in_=o_ps[b][DH:DH + 1, :])
            recip = npool.tile([1, 2, 512], F32, tag="rc", name="recip")
            nc.vector.reciprocal_approx_fast(out=recip, in_=den)
            rb = npool.tile([DH, 2, 512], F32, tag="rb", name="rb")
            nc.gpsimd.partition_broadcast(
                rb.rearrange("p a c -> p (a c)"),
                recip.rearrange("p a c -> p (a c)"), channels=DH)
            for b in range(B):
                nc.vector.tensor_mul(
                    attnT[h][64 * b:64 * b + 64, s * 512:(s + 1) * 512],
                    o_ps[b][0:DH, :], rb[:, b, :])
                nc.sync.dma_start(
                    out=a2a_in[h][4 * b + s],
                    in_=attnT[h][64 * b:64 * b + 64, s * 512:(s + 1) * 512])

        def exchange(h):
            nc.gpsimd.collective_compute(
                "AllToAll", mybir.AluOpType.bypass,
                replica_groups=[list(range(8))],
                ins=[a2a_in[h].ap().opt()], outs=[a2a_out[h].ap().opt()],
            )

        def gather(h):
            nc.sync.dma_start(
                out=g_sb[h * DH:(h + 1) * DH, :, :],
                in_=a2a_out[h].ap().rearrange("kt p c -> p kt c"))

        # emission ---------------------------------------------------------
        # Span-major, head 0 first within each span: h0 finishes its last
        # span one full h1-span before the end, so AllToAll(0) overlaps
        # h1's final span. proj(s+1) units are spread through span-s steps.
        def post_h0():
            exchange(0)
            gather(0)

        def post_h1():
            exchange(1)
            gather(1)

        for b in range(B):
            proj_qk(0, b, wq_bf, qTh)
            proj_qk(0, b, wk_bf, kTh)
            proj_v(0, b)

        # Block order: h0's spans front-loaded so AllToAll(0) completes
        # before attention ends; h1's late spans provide the cover.
        blocks = [(0, 0), (1, 0), (0, 1), (1, 1), (0, 2), (1, 2), (0, 3),
                  (1, 3)]
        # proj(s+1) filler units go into the blocks processed before the
        # first block that needs span s+1.
        fill_plan = {1: (0, 2), 2: (2, 4), 3: (4, 6)}  # span -> block range

        steps = []
        fill_at = {}
        bounds = []
        for (h, s) in blocks:
            base = len(steps)
            post = post_h0 if (h, s) == (0, NSP - 1) else (
                post_h1 if (h, s) == (1, NSP - 1) else None)
            steps += list(attention_span(h, s, post))
            bounds.append((base, len(steps)))
        for sp, (b0_, b1_) in fill_plan.items():
            lo = bounds[b0_][0]
            hi = bounds[b1_ - 1][1]
            units = proj_units(sp)
            stride = max(1, (hi - lo) // len(units))
            for u, unit in enumerate(units):
                fill_at.setdefault(lo + min(u * stride, hi - lo - 1),
                                   []).append(unit)

        n = len(steps)
        w_box = {}
        last_w = None
        for i in range(n + DEPTH):
            if i < n:
                w_box[i] = steps[i][0]()
                if i == n - 1:
                    last_w = w_box[i]
            for unit in fill_at.get(i, ()):
                unit()
            if i >= DEPTH:
                steps[i - DEPTH][1](w_box.pop(i - DEPTH))

        if DEBUG:
            nc.sync.dma_start(out=dbg["dbg_qt"].ap(), in_=qTh[0])
            nc.sync.dma_start(out=dbg["dbg_kt"].ap(), in_=kTh[0])
            nc.sync.dma_start(
                out=dbg["dbg_vp"].ap(),
                in_=vp[0].rearrange("p a b c -> p (a b c)"))
            nc.sync.dma_start(out=dbg["dbg_at"].ap(), in_=attnT[0])
            nc.sync.dma_start(
                out=dbg["dbg_gsb"].ap(), in_=g_sb.rearrange("p a c -> p (a c)"))

        # ---- tail ---------------------------------------------------------
        # gathers emitted inside post_h0/post_h1 (see exchange callers).
        # warm-keeper matmuls: consume the LAST attention output so the
        # scheduler cannot hoist them into the attention phase; they bridge
        # the PE-idle window (normalize tail + AllToAll(1)) so HAM never
        # sees a >3.4us gap and out_proj runs at full clock.
        warm_ps = ps_v.tile([128, 512], F32, tag="qk", name="warm_ps")
        for i in range(48):
            nc.tensor.matmul(
                warm_ps, lhsT=wo_bf[:, i % KT, 0:128],
                rhs=attnT[1][:, 3 * 512:4 * 512], start=True, stop=True)

        # out_proj on the gathered [1024, 512] attn^T slice
        for sb in range(4):
            ps = ps_s.tile([128, 2, 512], F32, tag="mm", name="op_ps")
            for nt in range(2):
                for kt in range(KT):
                    nc.tensor.matmul(
                        ps[:, nt, :],
                        lhsT=g_sb[:, kt, sb * 128:(sb + 1) * 128],
                        rhs=wo_bf[:, kt, nt * 512:(nt + 1) * 512],
                        start=(kt == 0), stop=(kt == KT - 1))
            ot = opool.tile([128, D], F32, tag="ot")
            nc.vector.tensor_add(
                ot, ps.rearrange("p a c -> p (a c)"), bias_b)
            nc.sync.dma_start(
                out=out_t[sb * 128:(sb + 1) * 128, :], in_=ot)

    nc.compile()
    return nc


def shard_inputs(x, Wq, Wk, Wv, Wo, bo):
    """Full f32 inputs -> per-core in_maps (host-side bf16 + layout prep)."""
    x = np.asarray(x, dtype=np.float32)
    # x^T per batch, tiled: [B, KT, 128, S]
    xt = np.ascontiguousarray(
        x.transpose(0, 2, 1).reshape(B, KT, 128, S)).astype(ml_dtypes.bfloat16)

    def tile_w(w, cols):
        w = np.asarray(w, dtype=np.float32)[:, cols]
        return np.ascontiguousarray(
            w.reshape(KT, 128, w.shape[1]).transpose(1, 0, 2)
        ).astype(ml_dtypes.bfloat16)

    wo_full = tile_w(Wo, slice(None))
    bo = np.asarray(bo, dtype=np.float32).reshape(1, D)
    in_maps = []
    for c in range(N_CORES):
        cols = slice(c * DCOL, (c + 1) * DCOL)
        in_maps.append({
            "xt": xt,
            "wq": tile_w(Wq, cols),
            "wk": tile_w(Wk, cols),
            "wv": tile_w(Wv, cols),
            "wo": wo_full,
            "bo": bo,
        })
    return in_maps


def assemble_output(results):
    """Per-core out slices -> full [B, S, D]."""
    out = np.empty((B, S, D), dtype=np.float32)
    for c in range(N_CORES):
        b, sl = c // 4, c % 4
        out[b, sl * S_SLICE:(sl + 1) * S_SLICE, :] = results[c]["out"]
    return out


def kernel(x, Wq, Wk, Wv, Wo, bo):
    if "nc" not in _CACHE:
        _CACHE["nc"] = build()
    nc = _CACHE["nc"]
    in_maps = shard_inputs(x, Wq, Wk, Wv, Wo, bo)
    res = run_bass_kernel_spmd(nc, in_maps, core_ids=list(range(N_CORES)))
    return assemble_output(res.results)


# revision 33
# speedup vs baseline: 1.1356x; 1.0295x over previous
"""Multi-head causal attention on 8 Trainium2 NeuronCores.

Problem: x [2, 2048, 1024] f32; Wq/Wk/Wv [1024, 1024]; Wo [1024, 1024]; bo [1024].
  q/k/v = split_heads(x @ W*)  (16 heads, head_dim 64)
  scores = q k^T, causal mask, / sqrt(1024), softmax, out = (w v) @ Wo + bo

Sharding: tensor-parallel over heads, 8-way (Megatron-style): core c computes
heads {2c, 2c+1} for both batches, exchanges attention outputs with two 8-rank
AllToAlls (one per local head), applies the full out_proj on its gathered
[1024, 512] attn^T slice, and returns rows (batch c//4, 512*(c%4):+512).

Host-side prep: x is transposed and cast to bf16 on the host (layout
[B, KT, 128, S]), weights are pre-tiled bf16 — the device kernel does zero
transposes and zero dtype casts of inputs.

On-chip: attention is computed fully transposed (scores^T = K Q^T in [k, q]
layout). The two BATCHES are processed as concurrent PE row-tiles
(tile_position (0,0) / (64,0)) since head_dim=64 is half the array: batch-0
q/k live in partitions 0:64, batch-1 in 64:128, so each matmul pair runs at
full PE rate. Heads run sequentially so head 0's AllToAll overlaps head 1's
attention. Causal diagonal blocks are column-sliced (no wasted exp/matmul on
fully-masked columns). The softmax denominator rides as a 65th column of V
(ones-column trick); normalization uses a gpsimd partition-broadcast.
"""

from contextlib import ExitStack

import numpy as np
import ml_dtypes

import concourse.bass as bass
import concourse.tile as tile
from concourse import bacc, mybir
from concourse.bass_utils import run_bass_kernel_spmd
from concourse.masks import make_identity

F32 = mybir.dt.float32
BF16 = mybir.dt.bfloat16

N_CORES = 8
B = 2
S = 2048
D = 1024
H = 16
DH = 64
H_PER = 2              # heads per core
DCOL = H_PER * DH      # 128: projection output cols per core
KT = D // 128          # 8 contraction tiles
NSP = S // 512         # 4 q-spans per batch
S_SLICE = S // 4       # 512 output rows per core
SCALE = 1.0 / np.sqrt(np.float32(D))
DEPTH = 4              # attention software-pipeline depth

_CACHE = {}
DEBUG = False


def build():
    """Build the SPMD program (identical on all 8 cores)."""
    nc = bacc.Bacc("TRN2", target_bir_lowering=False, debug=False)

    xt_t = nc.dram_tensor("xt", [B, KT, 128, S], BF16, kind="ExternalInput")
    wq_t = nc.dram_tensor("wq", [128, KT, DCOL], BF16, kind="ExternalInput")
    wk_t = nc.dram_tensor("wk", [128, KT, DCOL], BF16, kind="ExternalInput")
    wv_t = nc.dram_tensor("wv", [128, KT, DCOL], BF16, kind="ExternalInput")
    wo_t = nc.dram_tensor("wo", [128, KT, D], BF16, kind="ExternalInput")
    bo_t = nc.dram_tensor("bo", [1, D], F32, kind="ExternalInput")
    out_t = nc.dram_tensor("out", [S_SLICE, D], F32, kind="ExternalOutput")

    if DEBUG:
        dbg = {
            "dbg_qt": nc.dram_tensor("dbg_qt", [128, S], BF16,
                                     kind="ExternalOutput"),
            "dbg_kt": nc.dram_tensor("dbg_kt", [128, S], BF16,
                                     kind="ExternalOutput"),
            "dbg_vp": nc.dram_tensor("dbg_vp", [128, S // 128 * H_PER * (DH + 1)],
                                     BF16, kind="ExternalOutput"),
            "dbg_at": nc.dram_tensor("dbg_at", [128, S], BF16,
                                     kind="ExternalOutput"),
            "dbg_w": nc.dram_tensor("dbg_w", [128, 2 * 512], BF16,
                                    kind="ExternalOutput"),
            "dbg_gsb": nc.dram_tensor("dbg_gsb", [128, KT * 512], BF16,
                                      kind="ExternalOutput"),
            "dbg_o": nc.dram_tensor("dbg_o", [DH + 1, 512], F32,
                                    kind="ExternalOutput"),
            "dbg_rb": nc.dram_tensor("dbg_rb", [DH, 512], F32,
                                     kind="ExternalOutput"),
        }

    warm_in = nc.dram_tensor("warm_in", [8, 16], F32)
    warm_out = nc.dram_tensor("warm_out", [8, 16], F32)
    a2a_in = [
        nc.dram_tensor(f"a2a_in{h}", [8, DH, 512], BF16) for h in range(H_PER)
    ]
    a2a_out = [
        nc.dram_tensor(f"a2a_out{h}", [8, DH, 512], BF16) for h in range(H_PER)
    ]

    with tile.TileContext(nc) as tc, ExitStack() as ctx:
        const = ctx.enter_context(tc.tile_pool(name="const", bufs=1))
        persist = ctx.enter_context(tc.tile_pool(name="persist", bufs=1))
        vstage = ctx.enter_context(tc.tile_pool(name="vstage", bufs=2))
        wpool = ctx.enter_context(tc.tile_pool(name="wpool", bufs=6))
        npool = ctx.enter_context(tc.tile_pool(name="npool", bufs=2))
        opool = ctx.enter_context(tc.tile_pool(name="opool", bufs=3))
        ps_s = ctx.enter_context(tc.tile_pool(name="ps_s", bufs=2, space="PSUM"))
        ps_o = ctx.enter_context(tc.tile_pool(name="ps_o", bufs=2, space="PSUM"))
        ps_v = ctx.enter_context(tc.tile_pool(name="ps_v", bufs=1, space="PSUM"))

        # warmup collective: absorbs the per-execution ncfw entry cost
        # concurrently with the load/projection phase.
        nc.gpsimd.collective_compute(
            "AllToAll", mybir.AluOpType.bypass,
            replica_groups=[list(range(8))],
            ins=[warm_in.ap().opt()], outs=[warm_out.ap().opt()],
        )

        identity = const.tile([128, 128], BF16)
        make_identity(nc, identity)

        # persistent tiles -------------------------------------------------
        xts = [persist.tile([128, KT, S], BF16, tag=f"xts{b}", name=f"xts{b}")
               for b in range(B)]
        # q^T/k^T per head: rows 0:64 = batch 0, rows 64:128 = batch 1
        qTh = [persist.tile([128, S], BF16, tag=f"qT{h}", name=f"qT{h}")
               for h in range(H_PER)]
        kTh = [persist.tile([128, S], BF16, tag=f"kT{h}", name=f"kT{h}")
               for h in range(H_PER)]
        # v per batch: [kpos-part, sb, head, 65] (col 64 = ones -> denom)
        vp = [persist.tile([128, S // 128, H_PER, DH + 1], BF16,
                           tag=f"vp{b}", name=f"vp{b}") for b in range(B)]
        # attn^T per head: rows 0:64 = batch 0, 64:128 = batch 1
        attnT = [persist.tile([128, S], BF16, tag=f"attnT{h}",
                              name=f"attnT{h}") for h in range(H_PER)]
        wq_bf = persist.tile([128, KT, DCOL], BF16, tag="wq", name="wq")
        wk_bf = persist.tile([128, KT, DCOL], BF16, tag="wk", name="wk")
        wv_bf = persist.tile([128, KT, DCOL], BF16, tag="wv", name="wv")
        wo_bf = persist.tile([128, KT, D], BF16, tag="wo", name="wo")
        bias_b = persist.tile([128, D], F32, tag="bias", name="bias")
        g_sb = persist.tile([128, KT, 512], BF16, tag="g", name="g_sb")

        for b in range(B):
            nc.gpsimd.memset(vp[b][:, :, :, DH:], 1.0)

        # input DMAs -------------------------------------------------------
        nc.sync.dma_start(out=wq_bf, in_=wq_t.ap())
        nc.sync.dma_start(out=wk_bf, in_=wk_t.ap())
        nc.sync.dma_start(out=wv_bf, in_=wv_t.ap())
        # x^T chunks: 2KB per-partition lines (1024 tokens) for full DMA
        # bandwidth, span-pair-major so early spans land first
        for sp in range(2):
            for b in range(B):
                for kt in range(KT):
                    nc.sync.dma_start(
                        out=xts[b][:, kt, sp * 1024:(sp + 1) * 1024],
                        in_=xt_t[b, kt, :, sp * 1024:(sp + 1) * 1024])
        nc.sync.dma_start(out=wo_bf, in_=wo_t.ap())
        nc.sync.dma_start(out=bias_b, in_=bo_t[0:1, :].to_broadcast([128, D]))

        # projections ------------------------------------------------------
        def proj_qk(s, b, w_bf, dest):
            """One Q-or-K projection for (span s, batch b). psum rows
            0:64 = head 0, 64:128 = head 1; evicted into the per-head
            tiles at the batch's partition half."""
            cols = slice(s * 512, (s + 1) * 512)
            ps = ps_v.tile([128, 512], F32, tag="qk", name="qk_ps")
            for kt in range(KT):
                nc.tensor.matmul(
                    ps, lhsT=w_bf[:, kt, :], rhs=xts[b][:, kt, cols],
                    start=(kt == 0), stop=(kt == KT - 1))
            for h in range(H_PER):
                nc.vector.tensor_copy(
                    out=dest[h][64 * b:64 * b + 64, cols],
                    in_=ps[64 * h:64 * h + 64, :])

        def proj_v(s, b):
            """V^T projection + PE-transpose into AV layout."""
            cols = slice(s * 512, (s + 1) * 512)
            ps = ps_v.tile([128, 512], F32, tag="qk", name="vt_ps")
            for kt in range(KT):
                nc.tensor.matmul(
                    ps, lhsT=wv_bf[:, kt, :], rhs=xts[b][:, kt, cols],
                    start=(kt == 0), stop=(kt == KT - 1))
            vT = vstage.tile([128, 512], BF16, tag="vT", name="vT")
            nc.vector.tensor_copy(out=vT, in_=ps)
            pt = ps_v.tile([128, 4, 128], BF16, tag="pt", name="pt")
            for c in range(4):
                nc.tensor.transpose(
                    pt[:, c, :], vT[:, c * 128:(c + 1) * 128], identity)
            nc.vector.tensor_copy(
                out=vp[b][:, s * 4:s * 4 + 4, :, 0:DH],
                in_=pt.rearrange("p c (h d) -> p c h d", d=DH))

        def proj_units(s):
            return [
                lambda b=b: proj_qk(s, b, wq_bf, qTh) for b in range(B)
            ] + [
                lambda b=b: proj_qk(s, b, wk_bf, kTh) for b in range(B)
            ] + [
                lambda b=b: proj_v(s, b) for b in range(B)
            ]

        # attention --------------------------------------------------------
        def attention_span(h, s, post_done):
            """(emit_scores, emit_av) closure pairs for (head h, span s),
            batches 0/1 as concurrent PE row-tiles."""
            if True:
                last = 4 * s + 3
                o_ps = [ps_o.tile([DH + 1, 512], F32, tag="o", name=f"o{b}")
                        for b in range(B)]

                def mk_scores(s=s, kb=None):
                    def emit(kb=kb):
                        off = max(0, 128 * (kb - 4 * s))
                        sp = ps_s.tile([128, 2, 512], F32, tag="mm",
                                       name="s_ps")
                        for b in range(B):
                            nc.tensor.matmul(
                                sp[:, b, off:512],
                                lhsT=kTh[h][64 * b:64 * b + 64,
                                            kb * 128:(kb + 1) * 128],
                                rhs=qTh[h][64 * b:64 * b + 64,
                                           s * 512 + off:(s + 1) * 512],
                                start=True, stop=True)
                        w_bf = wpool.tile([128, 2, 512], BF16, tag="w",
                                          name="w_bf")
                        nc.scalar.activation(
                            w_bf[:, :, off:512], sp[:, :, off:512],
                            mybir.ActivationFunctionType.Exp,
                            scale=float(SCALE))
                        if DEBUG and h == 0 and s == 1 and kb == 0:
                            nc.sync.dma_start(
                                out=dbg["dbg_w"].ap(),
                                in_=w_bf.rearrange("p a c -> p (a c)"))
                        if kb >= 4 * s:
                            # causal: inside the diagonal 128-col block,
                            # keep iff col - kpos >= 0 (both batches at once)
                            nc.gpsimd.affine_select(
                                out=w_bf[:, :, off:off + 128],
                                in_=w_bf[:, :, off:off + 128],
                                pattern=[[0, 2], [1, 128]],
                                compare_op=mybir.AluOpType.is_ge,
                                fill=0.0, base=0, channel_multiplier=-1)
                        return w_bf
                    return emit

                def mk_av(s=s, kb=None, last=last, o_ps=o_ps):
                    def emit(w_bf, kb=kb):
                        off = max(0, 128 * (kb - 4 * s))
                        for b in range(B):
                            nc.tensor.matmul(
                                o_ps[b][:, off:512],
                                lhsT=vp[b][:, kb, h, :],
                                rhs=w_bf[:, b, off:512],
                                start=(kb == 0), stop=(kb == last),
                                skip_group_check=True)
                        if kb == last:
                            normalize(h, s, o_ps)
                            if post_done is not None:
                                post_done()
                    return emit

                for kb in range(last + 1):
                    yield mk_scores(kb=kb), mk_av(kb=kb)

        def normalize(h, s, o_ps):
            """Both batches at once: one reciprocal + one broadcast."""
            den = npool.tile([1, 2, 512], F32, tag="dn", name="den")
            for b in range(B):
                nc.vector.tensor_copy(out=den[:, b, :],
                                      in_=o_ps[b][DH:DH + 1, :])
            recip = npool.tile([1, 2, 512], F32, tag="rc", name="recip")
            nc.vector.reciprocal_approx_fast(out=recip, in_=den)
            rb = npool.tile([DH, 2, 512], F32, tag="rb", name="rb")
            nc.gpsimd.partition_broadcast(
                rb.rearrange("p a c -> p (a c)"),
                recip.rearrange("p a c -> p (a c)"), channels=DH)
            for b in range(B):
                nc.vector.tensor_mul(
                    attnT[h][64 * b:64 * b + 64, s * 512:(s + 1) * 512],
                    o_ps[b][0:DH, :], rb[:, b, :])
                nc.sync.dma_start(
                    out=a2a_in[h][4 * b + s],
                    in_=attnT[h][64 * b:64 * b + 64, s * 512:(s + 1) * 512])

        def exchange(h):
            nc.gpsimd.collective_compute(
                "AllToAll", mybir.AluOpType.bypass,
                replica_groups=[list(range(8))],
                ins=[a2a_in[h].ap().opt()], outs=[a2a_out[h].ap().opt()],
            )

        def gather(h):
            nc.sync.dma_start(
                out=g_sb[h * DH:(h + 1) * DH, :, :],
                in_=a2a_out[h].ap().rearrange("kt p c -> p kt c"))

        # emission ---------------------------------------------------------
        # Span-major, head 0 first within each span: h0 finishes its last
        # span one full h1-span before the end, so AllToAll(0) overlaps
        # h1's final span. proj(s+1) units are spread through span-s steps.
        def post_h0():
            exchange(0)

        def post_h1():
            exchange(1)

        for b in range(B):
            proj_qk(0, b, wq_bf, qTh)
            proj_qk(0, b, wk_bf, kTh)
            proj_v(0, b)

        # Block order: h0's spans front-loaded so AllToAll(0) completes
        # before attention ends; h1's late spans provide the cover.
        blocks = [(0, 0), (1, 0), (0, 1), (1, 1), (0, 2), (1, 2), (0, 3),
                  (1, 3)]
        # proj(s+1) filler units go into the blocks processed before the
        # first block that needs span s+1.
        fill_plan = {1: (0, 2), 2: (2, 4), 3: (4, 6)}  # span -> block range

        steps = []
        fill_at = {}
        bounds = []
        for (h, s) in blocks:
            base = len(steps)
            post = post_h0 if (h, s) == (0, NSP - 1) else (
                post_h1 if (h, s) == (1, NSP - 1) else None)
            steps += list(attention_span(h, s, post))
            bounds.append((base, len(steps)))
        for sp, (b0_, b1_) in fill_plan.items():
            lo = bounds[b0_][0]
            hi = bounds[b1_ - 1][1]
            units = proj_units(sp)
            stride = max(1, (hi - lo) // len(units))
            for u, unit in enumerate(units):
                fill_at.setdefault(lo + min(u * stride, hi - lo - 1),
                                   []).append(unit)

        n = len(steps)
        w_box = {}
        last_w = None
        for i in range(n + DEPTH):
            if i < n:
                w_box[i] = steps[i][0]()
                if i == n - 1:
                    last_w = w_box[i]
            for unit in fill_at.get(i, ()):
                unit()
            if i >= DEPTH:
                steps[i - DEPTH][1](w_box.pop(i - DEPTH))

        if DEBUG:
            nc.sync.dma_start(out=dbg["dbg_qt"].ap(), in_=qTh[0])
            nc.sync.dma_start(out=dbg["dbg_kt"].ap(), in_=kTh[0])
            nc.sync.dma_start(
                out=dbg["dbg_vp"].ap(),
                in_=vp[0].rearrange("p a b c -> p (a b c)"))
            nc.sync.dma_start(out=dbg["dbg_at"].ap(), in_=attnT[0])
            nc.sync.dma_start(
                out=dbg["dbg_gsb"].ap(), in_=g_sb.rearrange("p a c -> p (a c)"))

        # ---- tail ---------------------------------------------------------
        # gathers emitted HERE, after the last a2a chunk DMAs, so a gather
        # waiting on AllToAll(0) can't head-of-line-block the final chunk
        # DMAs on the Sync queue (that delayed the AllToAll(1) trigger ~11us).
        gather(0)
        gather(1)

        # warm-keeper matmuls: consume the LAST attention output so the
        # scheduler cannot hoist them into the attention phase; they bridge
        # the PE-idle window (normalize tail + AllToAll(1)) so HAM never
        # sees a >3.4us gap and out_proj runs at full clock.
        warm_ps = ps_v.tile([128, 512], F32, tag="qk", name="warm_ps")
        for i in range(48):
            nc.tensor.matmul(
                warm_ps, lhsT=wo_bf[:, i % KT, 0:128],
                rhs=attnT[1][:, 3 * 512:4 * 512], start=True, stop=True)

        # out_proj on the gathered [1024, 512] attn^T slice
        for sb in range(4):
            ps = ps_s.tile([128, 2, 512], F32, tag="mm", name="op_ps")
            for nt in range(2):
                for kt in range(KT):
                    nc.tensor.matmul(
                        ps[:, nt, :],
                        lhsT=g_sb[:, kt, sb * 128:(sb + 1) * 128],
                        rhs=wo_bf[:, kt, nt * 512:(nt + 1) * 512],
                        start=(kt == 0), stop=(kt == KT - 1))
            ot = opool.tile([128, D], F32, tag="ot")
            nc.vector.tensor_add(
                ot, ps.rearrange("p a c -> p (a c)"), bias_b)
            nc.sync.dma_start(
                out=out_t[sb * 128:(sb + 1) * 128, :], in_=ot)

    nc.compile()
    return nc


def shard_inputs(x, Wq, Wk, Wv, Wo, bo):
    """Full f32 inputs -> per-core in_maps (host-side bf16 + layout prep)."""
    x = np.asarray(x, dtype=np.float32)
    # x^T per batch, tiled: [B, KT, 128, S]
    xt = np.ascontiguousarray(
        x.transpose(0, 2, 1).reshape(B, KT, 128, S)).astype(ml_dtypes.bfloat16)

    def tile_w(w, cols):
        w = np.asarray(w, dtype=np.float32)[:, cols]
        return np.ascontiguousarray(
            w.reshape(KT, 128, w.shape[1]).transpose(1, 0, 2)
        ).astype(ml_dtypes.bfloat16)

    wo_full = tile_w(Wo, slice(None))
    bo = np.asarray(bo, dtype=np.float32).reshape(1, D)
    in_maps = []
    for c in range(N_CORES):
        cols = slice(c * DCOL, (c + 1) * DCOL)
        in_maps.append({
            "xt": xt,
            "wq": tile_w(Wq, cols),
            "wk": tile_w(Wk, cols),
            "wv": tile_w(Wv, cols),
            "wo": wo_full,
            "bo": bo,
        })
    return in_maps


def assemble_output(results):
    """Per-core out slices -> full [B, S, D]."""
    out = np.empty((B, S, D), dtype=np.float32)
    for c in range(N_CORES):
        b, sl = c // 4, c % 4
        out[b, sl * S_SLICE:(sl + 1) * S_SLICE, :] = results[c]["out"]
    return out


def kernel(x, Wq, Wk, Wv, Wo, bo):
    if "nc" not in _CACHE:
        _CACHE["nc"] = build()
    nc = _CACHE["nc"]
    in_maps = shard_inputs(x, Wq, Wk, Wv, Wo, bo)
    res = run_bass_kernel_spmd(nc, in_maps, core_ids=list(range(N_CORES)))
    return assemble_output(res.results)
